# revision 56
# baseline (speedup 1.0000x reference)
"""Multi-head attention (B=4, S=2048, D=1024, H=16) on 8 Trainium2 NeuronCores.

Sharding: core c handles batch c//2 and head-group c%2 (8 heads = 512 dims of
the per-head concat). Each core computes its q/k/v projections (tensor
parallel over heads), attention for its 8 heads, and a partial output
projection over its 512 concat dims; the host sums the two partials per batch.

Device dataflow (per core, all matmuls bf16 = full-rate PE with FWL weight
loads and background-buffer preload; fp32 PSUM accumulation):
  - qT/kT [d, s] layouts from projection (contraction over embedding on
    partitions), v in [s, d] layout with a fused ones-column per head.
  - scores computed transposed S^T[k, q]; the two heads of a pair are
    row-packed (tile_position rows 0/64) so they run concurrently on the PE.
  - exp(scale*s) is a single ACT op per k-chunk over both heads (the mask is
    all-ones per the problem spec; a numpy fallback handles anything else).
  - ctx^T = [V | 1]^T @ P^T accumulated over k-chunks; row 64 of the psum is
    the softmax denominator (flash-style deferred normalization).
  - normalization: reciprocal of the denominator row on DVE, partition
    broadcast on GpSimd (keeps the DVE queue and DMA rings out of the
    critical path), one DVE multiply per head into ctx^T.
  - out^T partial = ctx_cat^T chunks @ Wo^T slices, streamed to DRAM.

Host epilogue: out[b] = partial[2b] + partial[2b+1] + (Wo @ bv + bo); the
value bias commutes with softmax (rows sum to 1) so it is exact. The key bias
is softmax-invariant (constant per query) and is still applied on-device for
exactness; so is the query bias.
"""

import sys

sys.path.insert(0, "/opt/trn_rl_repo")

import numpy as np

import concourse.bacc as bacc
import concourse.mybir as mybir
import concourse.tile as tile
from concourse.bass_utils import run_bass_kernel_spmd

f32 = mybir.dt.float32
bf16 = mybir.dt.bfloat16
AF = mybir.ActivationFunctionType

B, S, E, H = 4, 2048, 1024, 16
DH = E // H  # 64
G = E // 2  # 512 dims per core (8 heads)
HL = H // 2  # heads per core
EC = E // 128  # 8 e-chunks (projection contraction)
DC = G // 128  # 4 head-pairs per core
QT = S // 512  # 4 q-tiles
KC = S // 128  # 16 k-chunks
GC = G // 128  # 4 chunks of the local concat dim (out-proj contraction)
SCALE = 1.0 / np.sqrt(np.float64(E))

_NC = None


def _build_program():
    nc = bacc.Bacc("TRN2", target_bir_lowering=False, debug=False, num_devices=8)

    xqT = nc.dram_tensor("xqT", [E, S], bf16, kind="ExternalInput").ap()
    xkT = nc.dram_tensor("xkT", [QT, 128, EC, 512], bf16, kind="ExternalInput").ap()
    xvT = nc.dram_tensor("xvT", [E, S], bf16, kind="ExternalInput").ap()
    wqT = nc.dram_tensor("wqT", [E, G], bf16, kind="ExternalInput").ap()
    wkT = nc.dram_tensor("wkT", [128, EC, G], bf16, kind="ExternalInput").ap()
    wvT = nc.dram_tensor("wvT", [E, G], bf16, kind="ExternalInput").ap()
    woT = nc.dram_tensor("woT", [G, E], bf16, kind="ExternalInput").ap()
    bqd = nc.dram_tensor("bqd", [128, DC], f32, kind="ExternalInput").ap()
    bkd = nc.dram_tensor("bkd", [128, DC], f32, kind="ExternalInput").ap()
    out = nc.dram_tensor("out", [E, S], f32, kind="ExternalOutput").ap()  # transposed

    def xstream(pool, src, lo, ncols, tag="xstream", eng=None):
        t = pool.tile([128, EC, 512], bf16, tag=tag)
        ap = src[:, lo : lo + ncols].rearrange("(ec p) s -> p ec s", p=128)
        (eng or nc.sync).dma_start(t[:, :, :ncols], ap)
        return t

    with tile.TileContext(nc) as tc:
        with (
            tc.tile_pool(name="weights", bufs=1) as wpool,
            tc.tile_pool(name="persist", bufs=1) as ppool,
            tc.tile_pool(name="stream", bufs=3) as stream,
            tc.tile_pool(name="qtile", bufs=2) as qpool,
        ):
            kT_sb = ppool.tile([128, DC, S], bf16)
            v_sb = ppool.tile([128, KC, HL, DH + 1], bf16)
            wq_sb = wpool.tile([128, EC, G], bf16)
            wo_sb = wpool.tile([128, GC, E], bf16)
            bq_sb = wpool.tile([128, DC], f32)
            bk_sb = wpool.tile([128, DC], f32)
            actwarm = wpool.tile([1, 8], f32)
            # ones column for the denominator fusion: preset whole tile, the
            # projection copies later overwrite cols 0..DH-1 of each head block
            nc.gpsimd.memset(v_sb[:], 1.0)
            # pull the exp table load off the attention critical path
            nc.gpsimd.memset(actwarm[:], 0.0)
            nc.scalar.activation(actwarm[:], actwarm[:], AF.Exp)
            # q-path inputs ride the otherwise-idle gpsimd ring so the qT0
            # projection at the end of phase 1 never waits on the sync chain
            nc.gpsimd.dma_start(bq_sb[:], bqd)
            nc.gpsimd.dma_start(wq_sb[:], wqT.rearrange("(ec p) g -> p ec g", p=128))

            # ---------------- phase 1: kT and v projections ----------------
            with (
                tc.tile_pool(name="wtmp", bufs=1) as wtmp,
                tc.tile_pool(name="vstream", bufs=2) as vstream,
                tc.tile_pool(name="pj_psum", bufs=4, space="PSUM") as pj_psum,
            ):
                wk_sb = wtmp.tile([128, EC, G], bf16)
                wv_sb = wtmp.tile([128, EC, G], bf16)
                nc.sync.dma_start(wk_sb[:], wkT)
                nc.sync.dma_start(bk_sb[:], bkd)

                def xkload(st):
                    t = stream.tile([128, EC, 512], bf16, tag="xk")
                    nc.sync.dma_start(t[:], xkT[st])
                    return t

                xk_ts = [xkload(0)]
                nc.sync.dma_start(
                    wv_sb[:], wvT.rearrange("(ec p) g -> p ec g", p=128)
                )
                for st in range(QT):
                    xk_t = xk_ts[0] if st == 0 else xkload(st)
                    for dc in range(DC):
                        ps = pj_psum.tile([128, 512], f32, tag="pj")
                        for ec in range(EC):
                            nc.tensor.matmul(
                                ps[:],
                                lhsT=wk_sb[:, ec, dc * 128 : (dc + 1) * 128],
                                rhs=xk_t[:, ec, :],
                                start=(ec == 0),
                                stop=(ec == EC - 1),
                            )
                        nc.vector.tensor_add(
                            out=kT_sb[:, dc, st * 512 : (st + 1) * 512],
                            in0=ps[:],
                            in1=bk_sb[:, dc : dc + 1].to_broadcast((128, 512)),
                        )

                for sg in range(S // 512):
                    xv_t = xstream(vstream, xvT, sg * 512, 512, tag="xvstream")
                    for sci in range(4):
                        sc = sg * 4 + sci
                        ps = pj_psum.tile([128, 512], f32, tag="pj")
                        for ec in range(EC):
                            nc.tensor.matmul(
                                ps[:, :G],
                                lhsT=xv_t[:, ec, sci * 128 : (sci + 1) * 128],
                                rhs=wv_sb[:, ec, :],
                                start=(ec == 0),
                                stop=(ec == EC - 1),
                            )
                        nc.vector.tensor_copy(
                            out=v_sb[:, sc, :, 0:DH],
                            in_=ps[:, :G].rearrange("p (h d) -> p h d", h=HL),
                        )
                nc.sync.dma_start(
                    wo_sb[:], woT.rearrange("(gc p) e -> p gc e", p=128)
                )

            # ---------------- phase 2: attention with interleaved proj/outproj ----------------
            ctxp_cm = tc.tile_pool(name="ctxp", bufs=1)
            ctxp = ctxp_cm.__enter__()
            ctxT_sb = ctxp.tile([128, DC, S], bf16)

            with (
                tc.tile_pool(name="exp", bufs=6) as epool,
                tc.tile_pool(name="norm", bufs=3) as npool,
                tc.tile_pool(name="outsb", bufs=4) as opool,
                tc.tile_pool(name="s_psum", bufs=2, space="PSUM") as s_psum,
                tc.tile_pool(name="c_psum", bufs=4, space="PSUM") as c_psum,
            ):
                qT_ts = {}
                xq_ts = {}

                def qproj_steps(qt, dc):
                    """one dc-chunk of the qT projection; yields every 2 matmuls"""
                    if dc == 0:
                        qT_ts[qt] = qpool.tile(
                            [128, DC, 512], bf16, tag="qT", name=f"qT{qt}"
                        )
                        xq_ts[qt] = xstream(
                        stream, xqT, qt * 512, 512,
                        eng=(nc.gpsimd if qt == 0 else nc.sync),
                    )
                    qT_t = qT_ts[qt]
                    ps = c_psum.tile([128, 512], f32, tag="ctx", name=f"qp{qt}_{dc}")
                    for ec in range(EC):
                        nc.tensor.matmul(
                            ps[:],
                            lhsT=wq_sb[:, ec, dc * 128 : (dc + 1) * 128],
                            rhs=xq_ts[qt][:, ec, :],
                            start=(ec == 0),
                            stop=(ec == EC - 1),
                        )
                        yield
                    nc.vector.tensor_add(
                        out=qT_t[:, dc, :],
                        in0=ps[:],
                        in1=bq_sb[:, dc : dc + 1].to_broadcast((128, 512)),
                    )

                def outproj_steps(st, ec):
                    """one ec-chunk of the transposed output projection; yields every 2 matmuls"""
                    ps = c_psum.tile([128, 512], f32, tag="ctx", name=f"op{st}_{ec}")
                    for gc in range(GC):
                        nc.tensor.matmul(
                            ps[:],
                            lhsT=wo_sb[:, gc, ec * 128 : (ec + 1) * 128],
                            rhs=ctxT_sb[:, gc, st * 512 : (st + 1) * 512],
                            start=(gc == 0),
                            stop=(gc == GC - 1),
                        )
                        yield
                    o_sb = opool.tile([128, 512], f32, tag="osb")
                    nc.vector.tensor_copy(out=o_sb[:], in_=ps[:])
                    nc.sync.dma_start(
                        out[ec * 128 : (ec + 1) * 128, st * 512 : (st + 1) * 512],
                        o_sb[:],
                    )

                def drive(bg, n=1):
                    """advance the background work queue by n yield-steps"""
                    while n > 0 and bg:
                        try:
                            next(bg[0])
                            n -= 1
                        except StopIteration:
                            bg.pop(0)

                for dc in range(DC):
                    for _ in qproj_steps(0, dc):
                        pass

                for qt in range(QT):
                    q0 = qt * 512
                    qT_t = qT_ts[qt]
                    for hp in range(DC):
                        bg = []
                        if qt < QT - 1:
                            bg.append(qproj_steps(qt + 1, hp))
                        if qt > 0:
                            bg.append(outproj_steps(qt - 1, 2 * hp))
                            bg.append(outproj_steps(qt - 1, 2 * hp + 1))
                        ctx0 = c_psum.tile([128, 512], f32, tag="ctx", name=f"c0_{qt}_{hp}")
                        ctx1 = c_psum.tile([128, 512], f32, tag="ctx", name=f"c1_{qt}_{hp}")
                        # software-pipelined: ctx(kc-1) and background work are
                        # emitted BEFORE the scores pair of kc so the scheduler
                        # keeps the two row-packed scores matmuls adjacent
                        pend = [None]

                        def ctx_pair(kc):
                            e = pend[0]
                            nc.tensor.matmul(
                                ctx0[0 : DH + 1, :],
                                lhsT=v_sb[:, kc, 2 * hp, :],
                                rhs=e[:, 0:512],
                                start=(kc == 0),
                                stop=(kc == KC - 1),
                            )
                            nc.tensor.matmul(
                                ctx1[0 : DH + 1, :],
                                lhsT=v_sb[:, kc, 2 * hp + 1, :],
                                rhs=e[:, 512:1024],
                                start=(kc == 0),
                                stop=(kc == KC - 1),
                            )

                        for kc in range(KC):
                            k0 = kc * 128
                            if kc > 0:
                                ctx_pair(kc - 1)
                            drive(bg, 1)
                            sp = s_psum.tile([128, 1024], f32, tag="sp")
                            nc.tensor.matmul(
                                sp[:, 0:512],
                                lhsT=kT_sb[0:64, hp, k0 : k0 + 128],
                                rhs=qT_t[0:64, hp, :],
                                start=True,
                                stop=True,
                            )
                            nc.tensor.matmul(
                                sp[:, 512:1024],
                                lhsT=kT_sb[64:128, hp, k0 : k0 + 128],
                                rhs=qT_t[64:128, hp, :],
                                start=True,
                                stop=True,
                            )
                            e = epool.tile([128, 1024], bf16, tag="exp")
                            nc.scalar.activation(e[:], sp[:], AF.Exp, scale=float(SCALE))
                            pend[0] = e
                        ctx_pair(KC - 1)
                        while bg:
                            drive(bg, 1)
                        # evacuate psum fast, then normalize in SBUF
                        for hq, cpsum in ((0, ctx0), (1, ctx1)):
                            pb = 64 * hq
                            qs = slice(q0, q0 + 512)
                            nc.vector.tensor_copy(
                                out=ctxT_sb[pb : pb + 64, hp, qs], in_=cpsum[0:DH, :]
                            )
                            den = npool.tile([1, 512], f32, tag="den")
                            nc.vector.tensor_copy(out=den[:], in_=cpsum[DH : DH + 1, :])
                            rec = npool.tile([1, 512], f32, tag="rec")
                            nc.vector.reciprocal_approx_fast(rec[:], den[:])
                            rb = npool.tile([128, 512], f32, tag="rb")
                            nc.gpsimd.partition_broadcast(rb[:], rec[:])
                            nc.vector.tensor_mul(
                                out=ctxT_sb[pb : pb + 64, hp, qs],
                                in0=ctxT_sb[pb : pb + 64, hp, qs],
                                in1=rb[pb : pb + 64, :],
                            )

                # tail: output projection for the last q-tile
                for ec in range(EC):
                    for _ in outproj_steps(QT - 1, ec):
                        pass
            ctxp_cm.__exit__(None, None, None)

    nc.compile()
    return nc


def _prep_core_inputs(query, key, value, Wq, bq, Wk, bk, Wv, Wo):
    """Per-core input maps: core c -> batch c//2, head-group c%2."""
    import ml_dtypes

    f = ml_dtypes.bfloat16
    maps = []
    for c in range(8):
        b, g = c // 2, c % 2
        lo = g * G
        maps.append(
            {
                "xqT": np.ascontiguousarray(query[b].T).astype(f, copy=False),
                "xkT": np.ascontiguousarray(
                    key[b].reshape(QT, 512, EC, 128).transpose(0, 3, 2, 1).astype(f, copy=False)
                ),
                "xvT": np.ascontiguousarray(value[b].T).astype(f, copy=False),
                "wqT": np.ascontiguousarray(Wq[lo : lo + G].T).astype(f, copy=False),
                "wkT": np.ascontiguousarray(
                    Wk[lo : lo + G].T.reshape(EC, 128, G).transpose(1, 0, 2).astype(f, copy=False)
                ),
                "wvT": np.ascontiguousarray(Wv[lo : lo + G].T).astype(f, copy=False),
                "woT": np.ascontiguousarray(Wo[:, lo : lo + G].T).astype(f, copy=False),
                "bqd": np.ascontiguousarray(bq[lo : lo + G].reshape(DC, 128).T).astype(np.float32),
                "bkd": np.ascontiguousarray(bk[lo : lo + G].reshape(DC, 128).T).astype(np.float32),
            }
        )
    return maps


def _numpy_reference(query, key, value, mask, Wq, bq, Wk, bk, Wv, bv, Wo, bo):
    """Exact numpy fallback (only used if mask is not all ones)."""
    q = (query @ Wq.T + bq).reshape(B, S, H, DH).transpose(0, 2, 1, 3)
    k = (key @ Wk.T + bk).reshape(B, S, H, DH).transpose(0, 2, 1, 3)
    v = (value @ Wv.T + bv).reshape(B, S, H, DH).transpose(0, 2, 1, 3)
    scores = np.einsum("bhqd,bhkd->bhqk", q, k) / np.sqrt(np.float32(E))
    m = mask[:, None, :, :]
    scores = np.where(m == 0, -np.inf, scores)
    scores -= scores.max(axis=-1, keepdims=True)
    p = np.exp(scores)
    p /= p.sum(axis=-1, keepdims=True)
    ctx = np.einsum("bhqk,bhkd->bhqd", p, v)
    concat = ctx.transpose(0, 2, 1, 3).reshape(B, S, E)
    return (concat @ Wo.T + bo).astype(np.float32)


def kernel(query, key, value, mask, Wq, bq, Wk, bk, Wv, bv, Wo, bo, _results=None):
    global _NC
    query = np.asarray(query, dtype=np.float32)
    key = np.asarray(key, dtype=np.float32)
    value = np.asarray(value, dtype=np.float32)
    mask = np.asarray(mask)
    Wq, bq = np.asarray(Wq, np.float32), np.asarray(bq, np.float32)
    Wk, bk = np.asarray(Wk, np.float32), np.asarray(bk, np.float32)
    Wv, bv = np.asarray(Wv, np.float32), np.asarray(bv, np.float32)
    Wo, bo = np.asarray(Wo, np.float32), np.asarray(bo, np.float32)

    if not np.all(mask == 1):
        return _numpy_reference(
            query, key, value, mask, Wq, bq, Wk, bk, Wv, bv, Wo, bo
        )

    if _NC is None:
        _NC = _build_program()
    in_maps = _prep_core_inputs(query, key, value, Wq, bq, Wk, bk, Wv, Wo)
    res = run_bass_kernel_spmd(_NC, in_maps, core_ids=list(range(8)))
    if _results is not None:
        _results.append(res)

    # host epilogue: sum the two head-group partials; bv commutes with softmax
    # (rows sum to 1) so its contribution is Wo @ bv, plus the output bias bo.
    extra = (Wo.astype(np.float64) @ bv.astype(np.float64) + bo.astype(np.float64)).astype(
        np.float32
    )
    out = np.empty((B, S, E), dtype=np.float32)
    for b in range(B):
        out[b] = (
            res.results[2 * b]["out"] + res.results[2 * b + 1]["out"]
        ).T + extra
    return out
